# revision 11
# baseline (speedup 1.0000x reference)
"""AttentionGate distributed Bass kernel for 8 TRN2 NeuronCores.

Reference computation (per batch sample b, with N = 64*64*64 spatial, C = 32 chans):
    w[i,j] = sum_n q[n,i] k[n,j]          [C, C]
    w      = softmax(w, axis=-1)
    att[n,i] = sum_j q[n,j] w[i,j]
    out    = q + att
Folding the residual:  out = q @ M  with  M[j,i] = delta_ji + w[i,j].

Sharding: data parallel over the flattened spatial axis (slice the D=64 axis
8 ways). Each core computes a partial [C, C] score matrix per batch over its
N-shard, a tiny per-batch AllReduce (4KB) combines them, softmax is computed
redundantly per core, and each core produces its own N-shard of the output.
The whole thing is a per-batch pipeline: batch b's AllReduce + attention
matmul + output DMA overlap batch b+1's input DMA + score matmuls, so only
the last batch's AllReduce latency is exposed.

Per-core layout: each batch's 32768 local rows are stored DRAM-contiguously as
[128 partitions, 256 rows, 32 chans] ("slab" layout: partition p owns rows
p*256..p*256+255).  Matmul 1 contracts n on partitions directly.

For matmul 2 (contraction over channels) we use the DVE 32x32 in-place block
transpose: qx[(a,c),(s,pl)] = q[32a+pl, s, c].  A block-diagonal stationary
Mblk[(a,c),(a',i)] = delta_aa' * M[c,i] then computes
    attx[(a',i),(s,pl)] = att[32a'+pl, s, i]
in one matmul per 512 columns (one PSUM bank), and a second DVE block
transpose of attx yields the natural [p, (s,i)] output layout directly.
No PE transposes, no grid rearrangement — the block-diagonal structure makes
the block-grid swap cancel.
"""

import sys
import numpy as np

sys.path.insert(0, "/opt/trn_rl_repo")

B = 4            # batch
C = 32           # channels
P = 128          # partitions
D = 64           # depth axis (sharded)
NCORES = 8
DLOC = D // NCORES           # 8 depth slices per core
NLOC = DLOC * 64 * 64        # 32768 rows per batch per core
SEG = NLOC // P              # 256 rows per partition slab
FREE = SEG * C               # 8192 f32 per partition per batch
NCHUNK = SEG                 # 256 [128,32] chunks per batch
SUB = 2                      # input DMA pieces per batch tensor
SUBF = FREE // SUB           # 4096 f32 per partition per piece
CH_PER_SUB = SUBF // C       # 128 score chunks per piece
OSUB = 4                     # output DMA pieces per batch
OSUBF = FREE // OSUB         # 2048 f32
GRP = 512                    # moving columns per att matmul (1 PSUM bank)

TRACE = False
LAST_RESULT = None

_nc_cache = {}


def _build_nc():
    if "nc" in _nc_cache:
        return _nc_cache["nc"]
    from concourse import bacc, tile, mybir

    f32 = mybir.dt.float32
    nc = bacc.Bacc(
        "TRN2", target_bir_lowering=False, debug=False, num_devices=NCORES
    )

    q_ext = nc.declare_dram_parameter("queries", [B, P, FREE], f32, isOutput=False)
    k_ext = nc.declare_dram_parameter("keys", [B, P, FREE], f32, isOutput=False)
    eye_ext = nc.declare_dram_parameter("eye128", [P, P], f32, isOutput=False)
    rep_ext = nc.declare_dram_parameter("rep32", [C, P], f32, isOutput=False)
    out_ext = nc.declare_dram_parameter("out", [B, P, FREE], f32, isOutput=True)

    with tile.TileContext(nc) as tc:
        with (
            tc.tile_pool(name="const", bufs=1) as cpool,
            tc.tile_pool(name="qx", bufs=3) as qxpool,
            tc.tile_pool(name="dram", bufs=1, space="DRAM") as dpool,
            tc.tile_pool(name="small", bufs=2) as spool,
            tc.tile_pool(name="stage", bufs=2) as stpool,
            tc.tile_pool(name="qin", bufs=2) as qpool,
            tc.tile_pool(name="kin", bufs=2) as kpool,
            tc.tile_pool(name="mblk", bufs=2) as mpool,
            tc.tile_pool(name="psc", bufs=2, space="PSUM") as pscores,
            tc.tile_pool(name="pm", bufs=2, space="PSUM") as pm,
            tc.tile_pool(name="pa", bufs=3, space="PSUM") as pa,
        ):
            eye_sb = cpool.tile([P, P], f32)
            nc.sync.dma_start(eye_sb[:], eye_ext[:])
            rep_sb = cpool.tile([C, P], f32)
            nc.sync.dma_start(rep_sb[:], rep_ext[:])

            # Warmup collective: absorbs the collectives entry barrier and
            # ncfw wakeup concurrently with pass-1 DMA/compute, so the real
            # AllReduces below run on a warm CC stream.
            cc_d_in = dpool.tile([C, C], f32, tag="ccdin", name="cc_d_in")
            cc_d_out = dpool.tile(
                [C, C], f32, addr_space="Shared", tag="ccdout", name="cc_d_out"
            )
            nc.gpsimd.dma_start(cc_d_in[:], q_ext[0, :C, :C])
            nc.gpsimd.collective_compute(
                "AllReduce",
                mybir.AluOpType.add,
                replica_groups=[list(range(NCORES))],
                ins=[cc_d_in[:].opt()],
                outs=[cc_d_out[:].opt()],
            )

            for h in range(B // 2):  # batch pairs
                qxs = []
                # ---------- pass 1 for the pair: scores + q block-transpose ----------
                cc_in = dpool.tile(
                    [2 * C, C], f32, tag=f"ccin{h}", name=f"cc_in{h}"
                )
                cc_out = dpool.tile(
                    [2 * C, C], f32, addr_space="Shared",
                    tag=f"ccout{h}", name=f"cc_out{h}",
                )
                for b2 in range(2):
                    b = 2 * h + b2
                    # qx[(a,c), (s,pl)] = q[32a+pl, s, c]
                    qx = qxpool.tile([P, FREE], f32, tag="qx")
                    qxs.append(qx)
                    sc_ps = pscores.tile([C, C], f32, tag="sc")
                    for j in range(SUB):
                        qtile = qpool.tile([P, SUBF], f32, tag="qin")
                        ktile = kpool.tile([P, SUBF], f32, tag="kin")
                        nc.sync.dma_start(
                            qtile[:], q_ext[b, :, j * SUBF : (j + 1) * SUBF]
                        )
                        nc.sync.dma_start(
                            ktile[:], k_ext[b, :, j * SUBF : (j + 1) * SUBF]
                        )
                        for s in range(CH_PER_SUB):
                            g = j * CH_PER_SUB + s
                            nc.tensor.matmul(
                                sc_ps[:, :],
                                qtile[:, s * C : (s + 1) * C],
                                ktile[:, s * C : (s + 1) * C],
                                start=(g == 0),
                                stop=(g == NCHUNK - 1),
                            )
                        # 32x32 block transpose of the whole piece (DVE)
                        nc.vector.transpose(
                            qx[:, j * SUBF : (j + 1) * SUBF], qtile[:]
                        )
                    sc_sb = spool.tile([C, C], f32, tag="scsb")
                    nc.scalar.copy(sc_sb[:], sc_ps[:])
                    nc.gpsimd.dma_start(cc_in[b2 * C : (b2 + 1) * C, :], sc_sb[:])
                # ---------- one AllReduce per pair (8KB over 8 cores) ----------
                nc.gpsimd.collective_compute(
                    "AllReduce",
                    mybir.AluOpType.add,
                    replica_groups=[list(range(NCORES))],
                    ins=[cc_in[:].opt()],
                    outs=[cc_out[:].opt()],
                )
                for b2 in range(2):
                    b = 2 * h + b2
                    qx = qxs[b2]
                    # ---------- softmax(b) on [32, 32] ----------
                    w_sb = spool.tile([C, C], f32, tag="wsb")
                    nc.gpsimd.dma_start(
                        w_sb[:], cc_out[b2 * C : (b2 + 1) * C, :]
                    )
                    mx = spool.tile([C, 1], f32, tag="mx")
                    nc.vector.tensor_reduce(
                        mx[:], w_sb[:], axis=mybir.AxisListType.X,
                        op=mybir.AluOpType.max,
                    )
                    wsh = spool.tile([C, C], f32, tag="wsh")
                    nc.vector.tensor_scalar(
                        wsh[:], w_sb[:], mx[:], None,
                        op0=mybir.AluOpType.subtract,
                    )
                    e_t = spool.tile([C, C], f32, tag="et")
                    ssum = spool.tile([C, 1], f32, tag="ssum")
                    nc.scalar.activation(
                        e_t[:], wsh[:], mybir.ActivationFunctionType.Exp,
                        accum_out=ssum[:],
                    )
                    rinv = spool.tile([C, 1], f32, tag="rinv")
                    nc.vector.reciprocal(rinv[:], ssum[:])
                    wsoft = spool.tile([C, C], f32, tag="wsoft")
                    nc.vector.tensor_scalar_mul(wsoft[:], e_t[:], rinv[:])
                    # wt[j, i] = wsoft[i, j] = w[b][i, j]  (one 32x32 DVE block)
                    wt = spool.tile([C, C], f32, tag="wt")
                    nc.vector.transpose(wt[:], wsoft[:])

                    # ---------- Mblk(b): block-diag I + w^T ----------
                    # mrep[(a,j), i] = wt[j, i];  rep32[j', a*32+j] = delta_j'j
                    mrep_ps = pm.tile([P, C], f32, tag="mrep")
                    nc.tensor.matmul(
                        mrep_ps[:], rep_sb[:], wt[:], start=True, stop=True
                    )
                    mblk = mpool.tile([P, P], f32, tag="mblk")
                    nc.scalar.copy(mblk[:], eye_sb[:])
                    for a in range(4):
                        nc.vector.tensor_tensor(
                            mblk[a * C : (a + 1) * C, a * C : (a + 1) * C],
                            eye_sb[a * C : (a + 1) * C, a * C : (a + 1) * C],
                            mrep_ps[a * C : (a + 1) * C, :],
                            op=mybir.AluOpType.add,
                        )

                    # ---------- pass 2(b): att = q @ M ----------
                    for piece in range(OSUB):
                        stage = stpool.tile([P, OSUBF], f32, tag="stage")
                        for g in range(OSUBF // GRP):
                            off = piece * OSUBF + g * GRP
                            att_ps = pa.tile([P, GRP], f32, tag="att")
                            nc.tensor.matmul(
                                att_ps[:],
                                mblk[:],
                                qx[:, off : off + GRP],
                                start=True,
                                stop=True,
                            )
                            # block-transpose straight out of PSUM into the
                            # output staging tile (natural layout)
                            nc.vector.transpose(
                                stage[:, g * GRP : (g + 1) * GRP], att_ps[:]
                            )
                        nc.sync.dma_start(
                            out_ext[b, :, piece * OSUBF : (piece + 1) * OSUBF],
                            stage[:],
                        )

    nc.compile()
    _nc_cache["nc"] = nc
    return nc


def _make_consts():
    eye = np.eye(P, dtype=np.float32)
    rep = np.tile(np.eye(C, dtype=np.float32), (1, 4))  # [32, 128]
    return eye, rep


def kernel(queries, keys):
    global LAST_RESULT
    from concourse.bass_utils import run_bass_kernel_spmd

    q = np.asarray(queries, dtype=np.float32)
    k = np.asarray(keys, dtype=np.float32)
    eye, rep = _make_consts()

    nc = _build_nc()

    in_maps = []
    for i in range(NCORES):
        qs = np.ascontiguousarray(q[:, i * DLOC : (i + 1) * DLOC]).reshape(
            B, P, FREE
        )
        ks = np.ascontiguousarray(k[:, i * DLOC : (i + 1) * DLOC]).reshape(
            B, P, FREE
        )
        in_maps.append({"queries": qs, "keys": ks, "eye128": eye, "rep32": rep})

    res = run_bass_kernel_spmd(
        nc, in_maps, core_ids=list(range(NCORES)), trace=TRACE
    )
    LAST_RESULT = res

    shards = []
    for i in range(NCORES):
        o = np.asarray(res.results[i]["out"]).reshape(B, DLOC, 64, 64, C)
        shards.append(o)
    return np.concatenate(shards, axis=1)


# revision 12
# speedup vs baseline: 1.1762x; 1.1762x over previous
"""AttentionGate distributed Bass kernel for 8 TRN2 NeuronCores.

Reference computation (per batch sample b, with N = 64*64*64 spatial, C = 32 chans):
    w[i,j] = sum_n q[n,i] k[n,j]          [C, C]
    w      = softmax(w, axis=-1)
    att[n,i] = sum_j q[n,j] w[i,j]
    out    = q + att
Folding the residual:  out = q @ M  with  M[j,i] = delta_ji + w[i,j].

Sharding: data parallel over the flattened spatial axis (slice the D=64 axis
8 ways). Each core computes a partial [C, C] score matrix per batch over its
N-shard, a per-batch-pair AllReduce (8KB over 8 cores) combines them, softmax
is computed redundantly per core, and each core produces its own N-shard of
the output.  Software-pipelined: both pairs' score passes and AllReduces are
issued before any attention pass, so engine FIFOs never head-of-line block a
later AllReduce behind post-AllReduce work, and a tiny warmup collective
absorbs the collectives entry barrier at kernel start.

Per-core layout: each batch's 32768 local rows are stored DRAM-contiguously as
[128 partitions, 256 rows, 32 chans] ("slab" layout: partition p owns rows
p*256..p*256+255).  Matmul 1 (scores) contracts n on partitions directly in
fp32.

For matmul 2 (contraction over channels) we use the DVE 32x32 in-place block
transpose of a bf16 copy of q: qx[(a,c),(s,pl)] = q[32a+pl, s, c].  A
block-diagonal bf16 stationary Mblk[(a,c),(a',i)] = delta_aa' * M[c,i] then
computes attx[(a',i),(s,pl)] = att[32a'+pl, s, i] in one matmul per 512
columns (one PSUM bank), and a second DVE block transpose of attx (fp32, out
of PSUM) yields the natural [p, (s,i)] output layout directly.  No PE
transposes and no grid rearrangement — the block-diagonal structure makes the
block-grid swap cancel.
"""

import sys
import numpy as np

sys.path.insert(0, "/opt/trn_rl_repo")

B = 4            # batch
C = 32           # channels
P = 128          # partitions
D = 64           # depth axis (sharded)
NCORES = 8
DLOC = D // NCORES           # 8 depth slices per core
NLOC = DLOC * 64 * 64        # 32768 rows per batch per core
SEG = NLOC // P              # 256 rows per partition slab
FREE = SEG * C               # 8192 f32 per partition per batch
NCHUNK = SEG                 # 256 [128,32] chunks per batch
SUB = 2                      # input DMA pieces per batch tensor
SUBF = FREE // SUB           # 4096 f32 per partition per piece
CH_PER_SUB = SUBF // C       # 128 score chunks per piece
OSUB = 4                     # output DMA pieces per batch
OSUBF = FREE // OSUB         # 2048 f32
GRP = 512                    # moving columns per att matmul (1 PSUM bank)

TRACE = False
LAST_RESULT = None

_nc_cache = {}


def _build_nc():
    if "nc" in _nc_cache:
        return _nc_cache["nc"]
    from concourse import bacc, tile, mybir

    f32 = mybir.dt.float32
    bf16 = mybir.dt.bfloat16
    nc = bacc.Bacc(
        "TRN2", target_bir_lowering=False, debug=False, num_devices=NCORES
    )

    q_ext = nc.declare_dram_parameter("queries", [B, P, FREE], f32, isOutput=False)
    k_ext = nc.declare_dram_parameter("keys", [B, P, FREE], f32, isOutput=False)
    eye_ext = nc.declare_dram_parameter("eye128", [P, P], f32, isOutput=False)
    rep_ext = nc.declare_dram_parameter("rep32", [C, P], f32, isOutput=False)
    out_ext = nc.declare_dram_parameter("out", [B, P, FREE], f32, isOutput=True)

    with tile.TileContext(nc) as tc:
        with (
            tc.tile_pool(name="const", bufs=1) as cpool,
            tc.tile_pool(name="qx", bufs=4) as qxpool,
            tc.tile_pool(name="qb16", bufs=2) as qbpool,
            tc.tile_pool(name="dram", bufs=1, space="DRAM") as dpool,
            tc.tile_pool(name="small", bufs=2) as spool,
            tc.tile_pool(name="stage", bufs=2) as stpool,
            tc.tile_pool(name="qin", bufs=2) as qpool,
            tc.tile_pool(name="kin", bufs=2) as kpool,
            tc.tile_pool(name="mblk", bufs=2) as mpool,
            tc.tile_pool(name="psc", bufs=2, space="PSUM") as pscores,
            tc.tile_pool(name="pm", bufs=2, space="PSUM") as pm,
            tc.tile_pool(name="pa", bufs=3, space="PSUM") as pa,
        ):
            eye_sb = cpool.tile([P, P], f32)
            nc.sync.dma_start(eye_sb[:], eye_ext[:])
            rep_sb = cpool.tile([C, P], f32)
            nc.sync.dma_start(rep_sb[:], rep_ext[:])

            # Warmup collective: absorbs the collectives entry barrier and
            # ncfw wakeup concurrently with pass-1 DMA/compute, so the real
            # AllReduces below run on a warm CC stream.
            cc_d_in = dpool.tile([C, C], f32, tag="ccdin", name="cc_d_in")
            cc_d_out = dpool.tile(
                [C, C], f32, addr_space="Shared", tag="ccdout", name="cc_d_out"
            )
            nc.gpsimd.dma_start(cc_d_in[:], q_ext[0, :C, :C])
            nc.gpsimd.collective_compute(
                "AllReduce",
                mybir.AluOpType.add,
                replica_groups=[list(range(NCORES))],
                ins=[cc_d_in[:].opt()],
                outs=[cc_d_out[:].opt()],
            )

            qx_all = []      # bf16 block-transposed q, one tile per batch
            cc_outs = []     # per-pair AllReduce outputs

            # ======== phase 1: scores + transposes + AllReduces ========
            for h in range(B // 2):  # batch pairs
                cc_in = dpool.tile(
                    [2 * C, C], f32, tag=f"ccin{h}", name=f"cc_in{h}"
                )
                cc_out = dpool.tile(
                    [2 * C, C], f32, addr_space="Shared",
                    tag=f"ccout{h}", name=f"cc_out{h}",
                )
                cc_outs.append(cc_out)
                for b2 in range(2):
                    b = 2 * h + b2
                    # qx[(a,c), (s,pl)] = q[32a+pl, s, c]  (bf16)
                    qx = qxpool.tile([P, FREE], bf16, tag="qx")
                    qx_all.append(qx)
                    sc_ps = pscores.tile([C, C], f32, tag="sc")
                    for j in range(SUB):
                        qtile = qpool.tile([P, SUBF], f32, tag="qin")
                        ktile = kpool.tile([P, SUBF], f32, tag="kin")
                        nc.sync.dma_start(
                            qtile[:], q_ext[b, :, j * SUBF : (j + 1) * SUBF]
                        )
                        nc.sync.dma_start(
                            ktile[:], k_ext[b, :, j * SUBF : (j + 1) * SUBF]
                        )
                        for s in range(CH_PER_SUB):
                            g = j * CH_PER_SUB + s
                            nc.tensor.matmul(
                                sc_ps[:, :],
                                qtile[:, s * C : (s + 1) * C],
                                ktile[:, s * C : (s + 1) * C],
                                start=(g == 0),
                                stop=(g == NCHUNK - 1),
                            )
                        # cast to bf16 (ScalarE), then 32x32 block transpose (DVE)
                        qb = qbpool.tile([P, SUBF], bf16, tag="qb16")
                        nc.scalar.copy(qb[:], qtile[:])
                        nc.vector.transpose(
                            qx[:, j * SUBF : (j + 1) * SUBF], qb[:]
                        )
                    sc_sb = spool.tile([C, C], f32, tag="scsb")
                    nc.scalar.copy(sc_sb[:], sc_ps[:])
                    nc.gpsimd.dma_start(cc_in[b2 * C : (b2 + 1) * C, :], sc_sb[:])
                # one AllReduce per pair (8KB over 8 cores)
                nc.gpsimd.collective_compute(
                    "AllReduce",
                    mybir.AluOpType.add,
                    replica_groups=[list(range(NCORES))],
                    ins=[cc_in[:].opt()],
                    outs=[cc_out[:].opt()],
                )

            # ======== phase 2: softmax + attention + output ========
            for h in range(B // 2):
                cc_out = cc_outs[h]
                for b2 in range(2):
                    b = 2 * h + b2
                    qx = qx_all[b]
                    # ---------- softmax(b) on [32, 32] ----------
                    w_sb = spool.tile([C, C], f32, tag="wsb")
                    nc.scalar.dma_start(
                        w_sb[:], cc_out[b2 * C : (b2 + 1) * C, :]
                    )
                    mx = spool.tile([C, 1], f32, tag="mx")
                    nc.vector.tensor_reduce(
                        mx[:], w_sb[:], axis=mybir.AxisListType.X,
                        op=mybir.AluOpType.max,
                    )
                    wsh = spool.tile([C, C], f32, tag="wsh")
                    nc.vector.tensor_scalar(
                        wsh[:], w_sb[:], mx[:], None,
                        op0=mybir.AluOpType.subtract,
                    )
                    e_t = spool.tile([C, C], f32, tag="et")
                    ssum = spool.tile([C, 1], f32, tag="ssum")
                    nc.scalar.activation(
                        e_t[:], wsh[:], mybir.ActivationFunctionType.Exp,
                        accum_out=ssum[:],
                    )
                    rinv = spool.tile([C, 1], f32, tag="rinv")
                    nc.vector.reciprocal(rinv[:], ssum[:])
                    wsoft = spool.tile([C, C], f32, tag="wsoft")
                    nc.vector.tensor_scalar_mul(wsoft[:], e_t[:], rinv[:])
                    # wt[j, i] = wsoft[i, j] = w[b][i, j]  (one 32x32 DVE block)
                    wt = spool.tile([C, C], f32, tag="wt")
                    nc.vector.transpose(wt[:], wsoft[:])

                    # ---------- Mblk(b): block-diag I + w^T (bf16) ----------
                    # mrep[(a,j), i] = wt[j, i];  rep32[j', a*32+j] = delta_j'j
                    mrep_ps = pm.tile([P, C], f32, tag="mrep")
                    nc.tensor.matmul(
                        mrep_ps[:], rep_sb[:], wt[:], start=True, stop=True
                    )
                    mblk = mpool.tile([P, P], bf16, tag="mblk")
                    nc.scalar.copy(mblk[:], eye_sb[:])
                    for a in range(4):
                        nc.vector.tensor_tensor(
                            mblk[a * C : (a + 1) * C, a * C : (a + 1) * C],
                            eye_sb[a * C : (a + 1) * C, a * C : (a + 1) * C],
                            mrep_ps[a * C : (a + 1) * C, :],
                            op=mybir.AluOpType.add,
                        )

                    # ---------- pass 2(b): att = q @ M ----------
                    for piece in range(OSUB):
                        stage = stpool.tile([P, OSUBF], f32, tag="stage")
                        for g in range(OSUBF // GRP):
                            off = piece * OSUBF + g * GRP
                            att_ps = pa.tile([P, GRP], f32, tag="att")
                            nc.tensor.matmul(
                                att_ps[:],
                                mblk[:],
                                qx[:, off : off + GRP],
                                start=True,
                                stop=True,
                            )
                            # block-transpose straight out of PSUM into the
                            # output staging tile (natural layout)
                            nc.vector.transpose(
                                stage[:, g * GRP : (g + 1) * GRP], att_ps[:]
                            )
                        nc.sync.dma_start(
                            out_ext[b, :, piece * OSUBF : (piece + 1) * OSUBF],
                            stage[:],
                        )

    nc.compile()
    _nc_cache["nc"] = nc
    return nc


def _make_consts():
    eye = np.eye(P, dtype=np.float32)
    rep = np.tile(np.eye(C, dtype=np.float32), (1, 4))  # [32, 128]
    return eye, rep


def kernel(queries, keys):
    global LAST_RESULT
    from concourse.bass_utils import run_bass_kernel_spmd

    q = np.asarray(queries, dtype=np.float32)
    k = np.asarray(keys, dtype=np.float32)
    eye, rep = _make_consts()

    nc = _build_nc()

    in_maps = []
    for i in range(NCORES):
        qs = np.ascontiguousarray(q[:, i * DLOC : (i + 1) * DLOC]).reshape(
            B, P, FREE
        )
        ks = np.ascontiguousarray(k[:, i * DLOC : (i + 1) * DLOC]).reshape(
            B, P, FREE
        )
        in_maps.append({"queries": qs, "keys": ks, "eye128": eye, "rep32": rep})

    res = run_bass_kernel_spmd(
        nc, in_maps, core_ids=list(range(NCORES)), trace=TRACE
    )
    LAST_RESULT = res

    shards = []
    for i in range(NCORES):
        o = np.asarray(res.results[i]["out"]).reshape(B, DLOC, 64, 64, C)
        shards.append(o)
    return np.concatenate(shards, axis=1)
